# revision 1
# baseline (speedup 1.0000x reference)
"""Distributed GQA attention block (B=2, S=2048, D=2048, H=16, KV=4, HD=128,
RoPE, causal) on 8 Trainium2 NeuronCores.

Sharding: tensor-parallel over heads. Core i computes q-heads {2i, 2i+1} and
kv-head i//2. Each core produces a partial output projection (its heads'
columns of wo); the host sums the 8 partials.

Q/K/V projections run as fp8e4 DoubleRow matmuls with both operands split
into hi+lo fp8 components; the three significant cross products
(hi*hi + hi*lo + lo*hi) reproduce bf16-or-better accuracy at 0.75x the
bf16 PE cycle count. Weights are pre-scaled by a power of two on the host
(fp8 dynamic-range centering) and the inverse scale is folded into the
PSUM evacuation. V is produced directly in natural [tk, hd] layout with x
as the stationary operand. Attention (scores, softmax, PV, output
projection) runs in bf16 with the transposed-strip dataflow: S^T = k^T.T q^T
per kv chunk, exp written directly into P^T strips, O accumulated via
[P^T | ones]-style extended PV matmuls (ones column of V accumulates the
softmax denominators), PE-transposed to O^T, and projected y^T = wo^T.T O^T.
"""

import math

import numpy as np
import ml_dtypes

B, S, D = 2, 2048, 2048
H, KV, HD = 16, 4, 128
NCORES = 8
HPC = H // NCORES  # q heads per core
THETA = 10000.0

ND = D // 128  # 128-deep contraction chunks
NT = S // 512  # 512-wide t-blocks per batch
NI = S // 128  # 128-wide tq/tk chunks per batch

KQ = 9  # wq (with 1/sqrt(HD) folded) fp8 pre-scale: 2^9
KK = 6  # wk/wv fp8 pre-scale: 2^6

_BUILD_CACHE = {}


def _split_multi_waits(nc, max_waits=1):
    """This walrus build rejects >1 sync wait per instruction. Move extra
    semaphore waits onto no-ops inserted before the instruction on the same
    engine."""
    import concourse.mybir as mybir

    n_split = 0
    for f in nc.m.functions:
        for bb in f.blocks:
            insts = bb.instructions
            i = 0
            while i < len(insts):
                inst = insts[i]
                si = getattr(inst, "sync_info", None)
                if si is not None and si.on_wait and len(si.on_wait) > max_waits:
                    waits = list(si.on_wait)
                    extra, keep = waits[:-max_waits], waits[-max_waits:]
                    si.on_wait = keep
                    inst.sync_info = si
                    for j, w in enumerate(extra):
                        noop = mybir.InstNoOp(
                            name=f"{inst.name}-wsplit{j}",
                            sync_info=mybir.SyncInfo(on_wait=[w], on_update=[]),
                            bass_nofuse=True,
                            engine=inst.engine,
                        )
                        try:
                            nc.register_instruction(noop, overwrite=True)
                        except Exception:
                            pass
                        insts.insert(i + j, noop)
                        n_split += 1
                    i += len(extra)
                i += 1
    return n_split


def _build():
    import concourse.bass as bass
    import concourse.mybir as mybir
    from concourse import tile
    from concourse.masks import make_identity, make_upper_triangular

    BF16, F32, F8 = mybir.dt.bfloat16, mybir.dt.float32, mybir.dt.float8e4
    MULT, ADD = mybir.AluOpType.mult, mybir.AluOpType.add
    EXP = mybir.ActivationFunctionType.Exp
    DR = mybir.MatmulPerfMode.DoubleRow

    nc = bass.Bass()
    # weights arrive pre-swizzled into the SBUF layout [128, ND*M] (2KB+
    # contiguous rows: full-rate DMA descriptors, no rearrange cost)
    xh_e = nc.declare_dram_parameter("xhiT", [B, D, S], F8, isOutput=False)
    xl_e = nc.declare_dram_parameter("xloT", [B, D, S], F8, isOutput=False)
    wq_h_e = nc.declare_dram_parameter("wqhiT", [128, ND * HPC * HD], F8, isOutput=False)
    wq_l_e = nc.declare_dram_parameter("wqloT", [128, ND * HPC * HD], F8, isOutput=False)
    wk_h_e = nc.declare_dram_parameter("wkhiT", [128, ND * HD], F8, isOutput=False)
    wk_l_e = nc.declare_dram_parameter("wkloT", [128, ND * HD], F8, isOutput=False)
    wv_h_e = nc.declare_dram_parameter("wvhiT", [128, ND * HD], F8, isOutput=False)
    wv_l_e = nc.declare_dram_parameter("wvloT", [128, ND * HD], F8, isOutput=False)
    wo_e = nc.declare_dram_parameter("woT", [HPC * HD, D], BF16, isOutput=False)
    cos_e = nc.declare_dram_parameter("cosT", [HD // 2, S], BF16, isOutput=False)
    sin_e = nc.declare_dram_parameter("sinT", [HD // 2, S], BF16, isOutput=False)
    yT_e = nc.declare_dram_parameter("yT", [D, B * S], BF16, isOutput=True)

    with tile.TileContext(nc) as tc:
        with (
            tc.tile_pool(name="const", bufs=1) as cpool,
            tc.tile_pool(name="w", bufs=1) as wpool,
            tc.tile_pool(name="x", bufs=1) as xpool,
            tc.tile_pool(name="act", bufs=1) as apool,
            tc.tile_pool(name="tmp", bufs=3) as tpool,
            tc.tile_pool(name="psA", bufs=6, space="PSUM") as psA,
            tc.tile_pool(name="psB", bufs=2, space="PSUM") as psB,
        ):
            # ---- weight / table loads (nc.sync HWDGE), interleaved with the
            # first x tiles so the PE can start within ~4us.
            wk_h = wpool.tile([128, ND, HD], F8, tag="wkh", name="wkh")
            wk_l = wpool.tile([128, ND, HD], F8, tag="wkl", name="wkl")
            wq_h = wpool.tile([128, ND, HPC * HD], F8, tag="wqh", name="wqh")
            wq_l = wpool.tile([128, ND, HPC * HD], F8, tag="wql", name="wql")
            wv_h = wpool.tile([128, ND, HD], F8, tag="wvh", name="wvh")
            wv_l = wpool.tile([128, ND, HD], F8, tag="wvl", name="wvl")

            def ld_w(t, e):
                nc.sync.dma_start(t[:], e.rearrange("p (d o) -> p d o", d=ND))

            xtiles = {}  # (hl, tb) -> [4 piece tiles], per current batch

            def ld_x(b, tb, hl):
                e = xh_e if hl == 0 else xl_e
                ts = []
                for pc in range(4):
                    t = xpool.tile(
                        [128, 4, 512], F8, tag=f"x{hl}{tb}{pc}",
                        name=f"x{hl}{tb}{pc}",
                    )
                    nc.sync.dma_start(
                        t[:],
                        e[
                            b,
                            pc * 512 : (pc + 1) * 512,
                            tb * 512 : (tb + 1) * 512,
                        ].rearrange("(d p) s -> p d s", p=128),
                    )
                    ts.append(t)
                xtiles[(hl, tb)] = ts

            # first k/q matmuls gate on these: thin, ordered loads. V runs as
            # a second pass so its weights load late; x tiles stream tb-major.
            # load order mirrors the A-first projection schedule: hi x tiles
            # and hi weights first, then the lo corrections, then tables/V/O.
            ld_w(wk_h, wk_h_e)
            ld_x(0, 0, 0)
            ld_w(wq_h, wq_h_e)
            ld_x(0, 1, 0)
            cosT = cpool.tile([HD, S], BF16, tag="cos", name="cos")
            nc.sync.dma_start(cosT[0:64, :], cos_e[:, :])
            sinT = cpool.tile([HD, S], BF16, tag="sin", name="sin")
            nc.sync.dma_start(sinT[0:64, :], sin_e[:, :])
            nc.vector.tensor_copy(cosT[64:128, :], cosT[0:64, :])
            nc.vector.tensor_scalar_mul(sinT[64:128, :], sinT[0:64, :], -1.0)
            ld_x(0, 0, 1)
            ld_w(wk_l, wk_l_e)
            ld_w(wq_l, wq_l_e)
            ld_x(0, 1, 1)
            ld_x(0, 2, 0)
            ld_x(0, 2, 1)
            ld_x(0, 3, 0)
            ld_x(0, 3, 1)
            ld_w(wv_h, wv_h_e)
            ld_w(wv_l, wv_l_e)
            wo_t = wpool.tile([128, HPC, D], BF16, tag="wot", name="wot")
            nc.sync.dma_start(wo_t[:], wo_e.rearrange("(c p) o -> p c o", p=128))

            ident = cpool.tile([128, 128], BF16, tag="ident", name="ident")
            make_identity(nc, ident[:])
            triu = cpool.tile([128, 128], BF16, tag="triu", name="triu")
            make_upper_triangular(nc, triu[:], val=1.0, diag=True)

            def dr3(acc, w_h, w_l, csl, tb):
                """acc += (W_h+W_l).T @ (x_h+x_l) over all ND chunks via
                3-product DoubleRow (drops lo*lo)."""
                xh, xl = xtiles[(0, tb)], xtiles[(1, tb)]
                ops = [(w_h, xh), (w_h, xl), (w_l, xh)]
                n = 0
                for wt, xt in ops:
                    for p in range(ND // 2):
                        lp = p % 2  # pair within the 4-chunk piece tile
                        nc.tensor.matmul(
                            acc[:],
                            wt[:, 2 * p : 2 * p + 2, csl],
                            xt[p // 2][:, 2 * lp : 2 * lp + 2, :],
                            start=(n == 0),
                            stop=(n == 3 * (ND // 2) - 1),
                            perf_mode=DR,
                        )
                        n += 1

            def rope(dst, acc, sl, scale):
                ev = tpool.tile([128, 512], BF16, tag="ropee", name="ropee", bufs=2)
                nc.scalar.mul(ev[:], acc[:], scale)
                sw = tpool.tile([128, 512], BF16, tag="ropesw", name="ropesw", bufs=2)
                nc.vector.tensor_copy(sw[0:64, :], ev[64:128, :])
                nc.vector.tensor_copy(sw[64:128, :], ev[0:64, :])
                t1 = tpool.tile([128, 512], BF16, tag="ropea", name="ropea", bufs=2)
                nc.vector.tensor_tensor(t1[:], sw[:], sinT[:, sl], op=MULT)
                t2 = tpool.tile([128, 512], BF16, tag="ropeb", name="ropeb", bufs=2)
                nc.vector.tensor_tensor(t2[:], ev[:], cosT[:, sl], op=MULT)
                nc.vector.tensor_tensor(dst[:, sl], t2[:], t1[:], op=ADD)

            for b in range(B):
                qTr = [
                    apool.tile([HD, S], BF16, tag=f"q{h}", name=f"q{h}")
                    for h in range(HPC)
                ]
                kTr = apool.tile([HD, S], BF16, tag="k", name="k")
                vnat = [
                    apool.tile([128, HD + 1], BF16, tag=f"vn{j}", name=f"vn{j}")
                    for j in range(NI)
                ]

                # ---- k/q projections ----
                # Per tb-pair: the six hi*hi product chains first (they only
                # need the hi x/w tiles, which arrive first), then the
                # hi*lo / lo*hi corrections + evacuations. Tracks the DMA
                # arrival order at startup so the PE starts within ~4us.
                def dr_part(acc, w_h, w_l, csl, tb, part):
                    xh, xl = xtiles[(0, tb)], xtiles[(1, tb)]
                    ops = [(w_h, xh)] if part == 0 else [(w_h, xl), (w_l, xh)]
                    n0 = 0 if part == 0 else ND // 2
                    n = n0
                    for wt, xt in ops:
                        for p in range(ND // 2):
                            lp = p % 2
                            nc.tensor.matmul(
                                acc[:],
                                wt[:, 2 * p : 2 * p + 2, csl],
                                xt[p // 2][:, 2 * lp : 2 * lp + 2, :],
                                start=(n == 0),
                                stop=(n == 3 * (ND // 2) - 1),
                                perf_mode=DR,
                            )
                            n += 1

                for tbp in range(NT // 2):
                    units = []
                    for tb in (2 * tbp, 2 * tbp + 1):
                        units.append((wk_h, wk_l, slice(0, HD), tb, kTr, 2.0**-KK))
                        for h in range(HPC):
                            units.append(
                                (wq_h, wq_l, slice(h * HD, (h + 1) * HD), tb,
                                 qTr[h], 2.0**-KQ)
                            )
                    if tbp == NT // 2 - 1:
                        # q ropes first on the final pair: the first scores
                        # strips consume qTr, k chunk 0 is already resident
                        units = units[1:3] + [units[0]] + units[4:6] + [units[3]]
                    accs = []
                    for w_h, w_l, csl, tb, dst, sc in units:
                        acc = psA.tile(
                            [128, 512], mybir.dt.float32, tag="acc", name="acc"
                        )
                        dr_part(acc, w_h, w_l, csl, tb, 0)
                        accs.append(acc)
                    for acc, (w_h, w_l, csl, tb, dst, sc) in zip(accs, units):
                        dr_part(acc, w_h, w_l, csl, tb, 1)
                        rope(dst, acc, slice(tb * 512, (tb + 1) * 512), sc)
                def vproj(j):
                    # V natural: x stationary, wv moving, per 128-wide t-chunk.
                    # Runs inside the attention loop (chunk j is first needed
                    # by pv(c=j) one block later) to fill the PE during the
                    # exp-heavy early strips.
                    tb, i = divmod(j, 4)
                    vacc = psA.tile(
                        [128, HD], mybir.dt.float32, tag="acc", name="vacc"
                    )
                    xh, xl = xtiles[(0, tb)], xtiles[(1, tb)]
                    tsl = slice(i * 128, (i + 1) * 128)
                    ops = [(xh, wv_h), (xl, wv_h), (xh, wv_l)]
                    n = 0
                    for xt, wt in ops:
                        for p in range(ND // 2):
                            lp = p % 2
                            nc.tensor.matmul(
                                vacc[:],
                                xt[p // 2][:, 2 * lp : 2 * lp + 2, tsl],
                                wt[:, 2 * p : 2 * p + 2, :],
                                start=(n == 0),
                                stop=(n == 3 * (ND // 2) - 1),
                                perf_mode=DR,
                            )
                            n += 1
                    # DVE evac: keeps ACT free for the exp stream
                    nc.vector.tensor_scalar_mul(vnat[j][:, 0:HD], vacc[:], 2.0**-KK)
                    nc.gpsimd.memset(vnat[j][:, HD : HD + 1], 1.0)


                # ---- attention: scores/exp, PV, and output projection
                # interleaved at strip granularity. After strip j both heads'
                # P^T rows for kv-chunks <= j exist, so PV column c=j runs
                # immediately; after every 4th column the corresponding
                # 512-wide slice of the output projection runs. This keeps PE
                # busy while ACT streams the exps.
                strips = {h: [] for h in range(HPC)}
                for h in range(HPC):
                    for j in range(NI):
                        strips[h].append(
                            apool.tile(
                                [128, S - j * 128],
                                BF16,
                                tag=f"pt{h}{j}",
                                name=f"pt{h}{j}",
                            )
                        )
                ots = [
                    apool.tile([128, S], BF16, tag=f"ot{h}", name=f"ot{h}")
                    for h in range(HPC)
                ]

                def scores(j, h):
                    wstrip = S - j * 128
                    pts = strips[h][j]
                    for c0 in range(0, wstrip, 512):
                        w = min(512, wstrip - c0)
                        sps = psA.tile([128, 512], mybir.dt.float32, tag="acc",
                                       name="s")
                        nc.tensor.matmul(
                            sps[:, :w],
                            kTr[:, j * 128 : (j + 1) * 128],
                            qTr[h][:, j * 128 + c0 : j * 128 + c0 + w],
                            start=True,
                            stop=True,
                        )
                        nc.scalar.activation(pts[:, c0 : c0 + w], sps[:, :w], EXP)
                        if c0 == 0:
                            nc.vector.tensor_tensor(
                                pts[:, 0:128], pts[:, 0:128], triu[:], op=MULT
                            )

                # pv stage 1: PV matmuls + kick the DVE normalize; stage 2 (one
                # strip later): PE transpose + copy into O^T, so the PE never
                # waits on the DVE chain.
                pv_osb = {}

                def pv_mm(c, h):
                    oext = psA.tile([128, 512], mybir.dt.float32, tag="acc",
                                    name="oext")
                    for j in range(c + 1):
                        nc.tensor.matmul(
                            oext[:, 0 : HD + 1],
                            strips[h][j][:, (c - j) * 128 : (c - j + 1) * 128],
                            vnat[j][:],
                            start=(j == 0),
                            stop=(j == c),
                        )
                    osb = tpool.tile([128, HD], BF16, tag=f"onat{h}",
                                     name="onat", bufs=3)
                    rcol = tpool.tile([128, 1], mybir.dt.float32, tag=f"rc{h}",
                                      name="rcol", bufs=3)
                    nc.vector.reciprocal(rcol[:], oext[:, HD : HD + 1])
                    nc.vector.tensor_scalar(
                        osb[:], oext[:, 0:HD], rcol[:], None, op0=MULT
                    )
                    pv_osb[(c, h)] = osb

                def pv_tp(c, h):
                    tp = psB.tile([128, 128], BF16, tag="pt", name="pt")
                    nc.tensor.transpose(tp[:], pv_osb.pop((c, h))[:], ident[:])
                    nc.vector.tensor_copy(ots[h][:, c * 128 : (c + 1) * 128], tp[:])

                def oproj_quad(cg, q, tailmode=False):
                    # Four dc-blocks share one [128, 4, 512] staging tile and
                    # one strided DMA (DGE cost is per instruction, not per
                    # byte). Quads are spread one-per-strip-block so the PSUM
                    # ring never sees a 16-tile demand burst.
                    tsl = slice(cg * 512, (cg + 1) * 512)
                    yrq = tpool.tile([128, 4, 512], BF16, tag="yrq",
                                     name="yrq", bufs=2)
                    for i in range(4):
                        dc = q * 4 + i
                        yps = psA.tile([128, 512], mybir.dt.float32,
                                       tag="acc", name="yps")
                        for oc in range(HPC):
                            nc.tensor.matmul(
                                yps[:],
                                wo_t[:, oc, dc * 128 : (dc + 1) * 128],
                                ots[oc][:, tsl],
                                start=(oc == 0),
                                stop=(oc == HPC - 1),
                            )
                        if i % (2 if tailmode else 4) == 1:
                            nc.scalar.copy(yrq[:, i, :], yps[:])
                        else:
                            nc.vector.tensor_copy(yrq[:, i, :], yps[:])
                    eng = nc.scalar if (tailmode and q % 2 == 1) else nc.gpsimd
                    eng.dma_start(
                        yT_e[q * 512 : (q + 1) * 512,
                             b * S + cg * 512 : b * S + (cg + 1) * 512]
                        .rearrange("(d p) s -> p d s", p=128),
                        yrq[:],
                    )

                vproj(0)
                vproj(1)
                quads = []
                for j in range(NI + 2):
                    if j < NI:
                        for h in range(HPC):
                            scores(j, h)
                    if j + 2 < NI:
                        vproj(j + 2)
                        # prefetch next batch's x for a tb once its last
                        # v-chunk has been issued (keeps WAR order correct)
                        if (j + 2) % 4 == 3 and b + 1 < B:
                            ld_x(b + 1, (j + 2) // 4, 0)
                            ld_x(b + 1, (j + 2) // 4, 1)
                    if 1 <= j <= NI:
                        for h in range(HPC):
                            pv_mm(j - 1, h)
                    if j >= 2:
                        c2 = j - 2
                        for h in range(HPC):
                            pv_tp(c2, h)
                        if c2 % 4 == 3:
                            quads.extend((c2 // 4, q) for q in range(4))
                        if c2 == NI - 1:
                            for cg, q in quads:
                                oproj_quad(cg, q, tailmode=(cg == NT - 1))
                            quads = []
                        elif quads:
                            oproj_quad(*quads.pop(0))

    _split_multi_waits(nc)
    nc.finalize()
    return nc


def _get_nc():
    if "nc" not in _BUILD_CACHE:
        _BUILD_CACHE["nc"] = _build()
    return _BUILD_CACHE["nc"]


def _prep_inputs(x, wq, wk, wv, wo):
    """Host-side shard prep: per-core transposed fp8 hi/lo weight splits and
    shared fp8 hi/lo x^T."""
    bf16 = ml_dtypes.bfloat16
    f8 = ml_dtypes.float8_e4m3

    xT = np.ascontiguousarray(np.transpose(x, (0, 2, 1)))
    xhi = xT.astype(f8)
    xlo = (xT - xhi.astype(np.float32)).astype(f8)

    # RoPE tables in [hd, s] layout; emb = concat([ang, ang]).
    inv_freq = 1.0 / (THETA ** (np.arange(0, HD, 2, dtype=np.float32) / HD))
    ang = np.arange(S, dtype=np.float32)[:, None] * inv_freq[None, :]  # [S, HD/2]
    cosT = np.cos(ang).T.astype(bf16)  # [HD/2, S]; device mirrors to 64..127
    sinT = (-np.sin(ang).T).astype(bf16)  # negated rows 0..63; device flips sign

    scale = 1.0 / math.sqrt(HD)

    def split(a, k):
        # a: [D, M] transposed weight; returns fp8 hi/lo pre-swizzled into
        # the device SBUF layout [128, ND*M] (partition p holds row d*128+p)
        s = np.ascontiguousarray(a * 2.0**k).astype(np.float32)
        hi = s.astype(f8)
        lo = (s - hi.astype(np.float32)).astype(f8)

        def swz(w):
            m = w.shape[1]
            return np.ascontiguousarray(
                w.reshape(ND, 128, m).transpose(1, 0, 2).reshape(128, ND * m)
            )

        return swz(hi), swz(lo)

    in_maps = []
    for c in range(NCORES):
        h0 = c * HPC
        g = (c * HPC) // (H // KV)
        wq_hi, wq_lo = split((wq[h0 * HD : (h0 + HPC) * HD, :] * scale).T, KQ)
        wk_hi, wk_lo = split(wk[g * HD : (g + 1) * HD, :].T, KK)
        wv_hi, wv_lo = split(wv[g * HD : (g + 1) * HD, :].T, KK)
        wo_c = wo[:, h0 * HD : (h0 + HPC) * HD].T  # [HPC*HD, D]
        in_maps.append(
            {
                "xhiT": xhi,
                "xloT": xlo,
                "wqhiT": wq_hi,
                "wqloT": wq_lo,
                "wkhiT": wk_hi,
                "wkloT": wk_lo,
                "wvhiT": wv_hi,
                "wvloT": wv_lo,
                "woT": np.ascontiguousarray(wo_c).astype(bf16),
                "cosT": cosT,
                "sinT": sinT,
            }
        )
    return in_maps


def _gather(results):
    acc = np.zeros((D, B * S), np.float32)
    for r in results:
        acc += r["yT"].astype(np.float32)
    return np.ascontiguousarray(acc.T).reshape(B, S, D)


def kernel(x, wq, wk, wv, wo):
    from concourse.bass_utils import run_bass_kernel_spmd

    # Coerce to host numpy: device-array inputs must not trigger on-device
    # host math in _prep_inputs.
    x = np.asarray(x, dtype=np.float32)
    wq = np.asarray(wq, dtype=np.float32)
    wk = np.asarray(wk, dtype=np.float32)
    wv = np.asarray(wv, dtype=np.float32)
    wo = np.asarray(wo, dtype=np.float32)

    nc = _get_nc()
    in_maps = _prep_inputs(x, wq, wk, wv, wo)
    res = run_bass_kernel_spmd(nc, in_maps, core_ids=list(range(NCORES)))
    return _gather(res.results)

